# revision 1
# baseline (speedup 1.0000x reference)
"""Gaussian L1-distance attention kernel for Trainium2 (8 NeuronCores).

Computes y[b,s,i,j] = exp(-(sum_d |x[b,i,d]-x[b,j,d]|)^2 / (2*sigma_s^2))
for x [4,2048,3] f32, sigmas [8] f32 -> y [4,8,2048,2048] f32 (512MB).

The distance matrix is symmetric, so each core computes only the upper
triangle (53% of the elements) and the host mirrors the lower triangle
during unsharding (bit-exact: |a-b| and the downstream ops are symmetric).

Sharding (SPMD-uniform): core c -> batch b=c//2, sigma half h=c%2.
Every core processes the same 16 row-tiles (row-tile r: rows
r*128..r*128+127, columns r*128..2047, width 128*(16-r)) for its 4
sigmas. Identical shapes/offsets on every core; only input data differs.
Tiles are processed small->large->small so pipeline fill and drain are
both short.

Engine split per tile (all sized to stay under the ~98us DMA floor):
- VectorE: |xb_d - xi_d| as tensor_scalar subtract then in-place
  bitwise_and 0x7fffffff on a uint32 bitcast (sign-bit clear; abs is not
  a valid DVE ALU op, and both ops run in 2x perf mode), plus a0+a1.
- GPSIMD:  dist = s01 + a2 (otherwise idle engine).
- ScalarE: square + 4 exps with per-partition scale AP (-1/(2 sigma^2)
  computed on-chip from the sigmas input).
- One combined DMA store per tile covering all 4 sigmas.

Built with Bacc (not Bass): its finalize() runs generate_event_semaphores,
which splits instructions carrying more than one sync wait (TRN2 ISA
allows a single wait per compute instruction).
"""

import numpy as np

B, N, D, S = 4, 2048, 3, 8
NCORES = 8
NTILES = 16
S_LOC = S // 2                       # 4 sigmas per core
# ascending widths: index k -> width 128*(k+1), row-tile r = 15-k
# processing order: small -> large -> small
PROC = [0, 2, 4, 6, 8, 10, 12, 14, 15, 13, 11, 9, 7, 5, 3, 1]
PW = [128 * (k + 1) for k in PROC]   # processed widths
PR = [15 - k for k in PROC]          # processed row-tiles
WOFF = [sum(PW[:j]) for j in range(NTILES)]
PACKW = sum(PW)                      # 17408
XI_OFF = D * N                       # 6144: xi_all [16*3] in PROC order
SIG_OFF = XI_OFF + NTILES * D        # 6192: this core's 4 sigmas
XC_W = SIG_OFF + S_LOC               # 6196

_cached = None
TRACE_KW: dict = {}
LAST_RESULT = None
GPS_ADD = True                       # dist-add on GPSIMD vs VectorE


def _build():
    from concourse import mybir
    from concourse.bacc import Bacc
    from concourse.tile import TileContext

    f32 = mybir.dt.float32
    u32 = mybir.dt.uint32
    Alu = mybir.AluOpType
    Act = mybir.ActivationFunctionType

    nc = Bacc()
    xc = nc.dram_tensor("xc", [128, XC_W], f32, kind="ExternalInput")
    y = nc.dram_tensor("y", [S_LOC, 128, PACKW], f32, kind="ExternalOutput")

    with TileContext(nc) as tc:
        with (
            tc.tile_pool(name="const", bufs=1) as cpool,
            tc.tile_pool(name="absd", bufs=4) as apool,
            tc.tile_pool(name="mid", bufs=2) as mpool,
            tc.tile_pool(name="distp", bufs=2) as dpool,
            tc.tile_pool(name="sqp", bufs=2) as qpool,
            tc.tile_pool(name="outp", bufs=2) as opool,
        ):
            xcs = cpool.tile([128, XC_W], f32)
            nc.sync.dma_start(out=xcs[:], in_=xc[:])
            sig = xcs[:, SIG_OFF:SIG_OFF + S_LOC]
            # neg_inv[:, sl] = -1/(2*sigma^2)
            s2 = cpool.tile([128, S_LOC], f32)
            nc.vector.tensor_tensor(out=s2[:], in0=sig, in1=sig, op=Alu.mult)
            s2n = cpool.tile([128, S_LOC], f32)
            nc.vector.tensor_scalar_mul(s2n[:], s2[:], -2.0)
            neg_inv = cpool.tile([128, S_LOC], f32)
            nc.vector.reciprocal(out=neg_inv[:], in_=s2n[:])

            for j in range(NTILES):
                w, r = PW[j], PR[j]
                c0 = r * 128
                aa = []
                for d in range(D):
                    a = apool.tile([128, w], f32, tag="a")
                    nc.vector.tensor_scalar(
                        a[:], xcs[:, d * N + c0:d * N + c0 + w],
                        xcs[:, XI_OFF + j * D + d:XI_OFF + j * D + d + 1],
                        None, Alu.subtract,
                    )
                    au = a[:].bitcast(u32)
                    nc.vector.tensor_scalar(au, au, 0x7FFFFFFF, None, Alu.bitwise_and)
                    aa.append(a)
                s01 = mpool.tile([128, w], f32, tag="s01")
                nc.vector.tensor_tensor(out=s01[:], in0=aa[0][:], in1=aa[1][:], op=Alu.add)
                dist = dpool.tile([128, w], f32, tag="dist")
                eng = nc.gpsimd if GPS_ADD else nc.vector
                eng.tensor_tensor(out=dist[:], in0=s01[:], in1=aa[2][:], op=Alu.add)
                sq = qpool.tile([128, w], f32, tag="sq")
                nc.scalar.square(out=sq[:], in_=dist[:])

                o = opool.tile([128, S_LOC * w], f32, tag="o")
                for sl in range(S_LOC):
                    nc.scalar.activation(
                        out=o[:, sl * w:(sl + 1) * w], in_=sq[:], func=Act.Exp,
                        scale=neg_inv[:, sl:sl + 1],
                    )
                nc.sync.dma_start(
                    out=y[:, :, WOFF[j]:WOFF[j] + w].rearrange("s p w -> p s w"),
                    in_=o[:].rearrange("p (s w) -> p s w", s=S_LOC),
                )
    nc.finalize()
    return nc


def _pack_core_input(xb: np.ndarray, sig4: np.ndarray) -> np.ndarray:
    """xb: [N, D] batch slice; sig4: this core's 4 sigmas."""
    out = np.empty((128, XC_W), dtype=np.float32)
    out[:, :XI_OFF] = xb.T.reshape(1, D * N)
    rows = xb.reshape(NTILES, 128, D)        # [r, p, d]
    rows = rows[PR]                          # [j, p, d] in PROC order
    out[:, XI_OFF:SIG_OFF] = rows.transpose(1, 0, 2).reshape(128, NTILES * D)
    out[:, SIG_OFF:] = sig4[None, :]
    return out


def kernel(x: np.ndarray, sigmas: np.ndarray) -> np.ndarray:
    global _cached, LAST_RESULT
    from concourse import bass_utils

    x = np.ascontiguousarray(np.asarray(x, dtype=np.float32))
    sigmas = np.ascontiguousarray(np.asarray(sigmas, dtype=np.float32))

    if _cached is None:
        _cached = _build()
    nc = _cached

    in_maps = []
    for c in range(NCORES):
        b, h = c // 2, c % 2
        in_maps.append({
            "xc": _pack_core_input(x[b], sigmas[h * S_LOC:(h + 1) * S_LOC]),
        })

    res = bass_utils.run_bass_kernel_spmd(
        nc, in_maps, core_ids=list(range(NCORES)), **TRACE_KW
    )
    LAST_RESULT = res

    out = np.empty((B, S, N, N), dtype=np.float32)
    for c in range(NCORES):
        b, h = c // 2, c % 2
        yl = res.results[c]["y"]             # [S_LOC, 128, PACKW]
        for j in range(NTILES):
            r, w = PR[j], PW[j]
            out[b, h * S_LOC:(h + 1) * S_LOC, r * 128:(r + 1) * 128, r * 128:] = (
                yl[:, :, WOFF[j]:WOFF[j] + w]
            )
    # mirror the lower triangle (bit-exact by symmetry)
    for r in range(NTILES - 1):
        src = out[:, :, r * 128:(r + 1) * 128, (r + 1) * 128:]
        out[:, :, (r + 1) * 128:, r * 128:(r + 1) * 128] = src.swapaxes(-1, -2)
    return out



# revision 3
# speedup vs baseline: 1.4064x; 1.4064x over previous
"""Gaussian L1-distance attention kernel for Trainium2 (8 NeuronCores).

Computes y[b,s,i,j] = exp(-(sum_d |x[b,i,d]-x[b,j,d]|)^2 / (2*sigma_s^2))
for x [4,2048,3] f32, sigmas [8] f32 -> y [4,8,2048,2048] f32 (512MB).

Symmetry: only the upper (block-)triangle (53%) is computed; the host
mirrors the lower triangle during unsharding (bit-exact: |a-b| symmetric).

Sharding: core c -> batch b=c//2, column-parity h=c%2. Each core handles
all 8 sigmas over every row-tile strip r (rows r*128..+128, cols
r*128..2047), taking the even (h=0) or odd (h=1) columns of the strip via
stride-2 reads of the broadcast x plane. For h=1 the host pre-shifts the
plane one column left so both parities read identical SBUF offsets (SPMD).
Per-core columns: 8704.

Row-tiles are packed into 4 groups of exactly 2176 columns ({r,15-r} pair
sums are constant) so the 8 exp activations per group run with a large
free dim, amortizing ScalarE's ~435-cycle instruction overhead.

Engine split per group:
- VectorE:  |xb_d - xi_d| (fused sub+abs if available, else sub then
  sign-mask), then s01 = a0+a1.
- GPSIMD:   dist = s01 + a2;  sq = dist*dist  (otherwise-idle engine).
- ScalarE:  8x activation(Exp, scale=-1/(2 sigma^2) per-partition AP),
  writing bf16 directly (rel err ~0.4% << 2e-2 gate).
- DMA:      one contiguous 557KB store per (group, sigma).

Output is bf16 (halves HBM write traffic vs f32); host upcasts to f32.
"""

import numpy as np

B, N, D, S = 4, 2048, 3, 8
NCORES = 8
NT = 16                               # row-tiles
HW = [64 * (16 - r) for r in range(NT)]   # per-core half-widths
GROUPS = [(0, 7, 8, 15), (1, 6, 9, 14), (2, 5, 10, 13), (3, 4, 11, 12)]
NG = len(GROUPS)
GW = 2176                             # group width (sum of HW over a group)
XI_OFF = D * N                        # 6144: xi [r*D + d] per partition
SIG_OFF = XI_OFF + NT * D             # 6192: 8 sigmas
XC_W = SIG_OFF + S                    # 6200

FUSED_ABS = False                     # ts(sub, bitwise_and-ptr) single pass
_cached = None
TRACE_KW: dict = {}
LAST_RESULT = None


def _build():
    from concourse import mybir
    from concourse.bacc import Bacc
    from concourse.tile import TileContext

    f32 = mybir.dt.float32
    u32 = mybir.dt.uint32
    Alu = mybir.AluOpType
    Act = mybir.ActivationFunctionType

    nc = Bacc()
    xc = nc.dram_tensor("xc", [128, XC_W], f32, kind="ExternalInput")
    y = nc.dram_tensor("y", [NG, S, 128, GW], mybir.dt.bfloat16,
                       kind="ExternalOutput")

    with TileContext(nc) as tc:
        with (
            tc.tile_pool(name="const", bufs=1) as cpool,
            tc.tile_pool(name="absd", bufs=2) as apool,
            tc.tile_pool(name="mid", bufs=2) as mpool,
            tc.tile_pool(name="distp", bufs=2) as dpool,
            tc.tile_pool(name="sqp", bufs=2) as qpool,
            tc.tile_pool(name="outp", bufs=4) as opool,
        ):
            xcs = cpool.tile([128, XC_W], f32)
            nc.sync.dma_start(out=xcs[:], in_=xc[:])
            sig = xcs[:, SIG_OFF:SIG_OFF + S]
            # neg_inv[:, s] = -1/(2*sigma_s^2)
            s2 = cpool.tile([128, S], f32)
            nc.vector.tensor_tensor(out=s2[:], in0=sig, in1=sig, op=Alu.mult)
            s2n = cpool.tile([128, S], f32)
            nc.vector.tensor_scalar_mul(s2n[:], s2[:], -2.0)
            neg_inv = cpool.tile([128, S], f32)
            nc.vector.reciprocal(out=neg_inv[:], in_=s2n[:])
            if FUSED_ABS:
                mask = cpool.tile([128, 1], u32)
                nc.gpsimd.memset(mask[:], 0x7FFFFFFF)

            for g, grp in enumerate(GROUPS):
                a = apool.tile([128, D * GW], f32, tag="a")
                off = 0
                for r in grp:
                    w = HW[r]
                    for d in range(D):
                        # xb plane window: stride-2 over [128r, 128r+2w)
                        src = xcs[:, d * N + r * 128:d * N + r * 128 + 2 * w:2]
                        xi = xcs[:, XI_OFF + r * D + d:XI_OFF + r * D + d + 1]
                        dst = a[:, d * GW + off:d * GW + off + w]
                        if FUSED_ABS:
                            nc.vector.tensor_scalar(
                                dst, src, xi, mask[:].bitcast(f32),
                                Alu.subtract, Alu.bitwise_and,
                            )
                        else:
                            nc.vector.tensor_scalar(
                                dst, src, xi, None, Alu.subtract)
                            du = dst.bitcast(u32)
                            nc.vector.tensor_scalar(
                                du, du, 0x7FFFFFFF, None, Alu.bitwise_and)
                    off += w
                s01 = mpool.tile([128, GW], f32, tag="s01")
                nc.vector.tensor_tensor(
                    out=s01[:], in0=a[:, 0:GW], in1=a[:, GW:2 * GW], op=Alu.add
                )
                dist = dpool.tile([128, GW], f32, tag="dist")
                nc.gpsimd.tensor_tensor(
                    out=dist[:], in0=s01[:], in1=a[:, 2 * GW:3 * GW], op=Alu.add
                )
                sq = qpool.tile([128, GW], f32, tag="sq")
                nc.gpsimd.tensor_tensor(
                    out=sq[:], in0=dist[:], in1=dist[:], op=Alu.mult
                )
                for s in range(S):
                    o = opool.tile([128, GW], mybir.dt.bfloat16, tag="o")
                    nc.scalar.activation(
                        out=o[:], in_=sq[:], func=Act.Exp,
                        scale=neg_inv[:, s:s + 1],
                    )
                    nc.sync.dma_start(out=y[g, s], in_=o[:])
    nc.finalize()
    return nc


def _pack_core_input(xb: np.ndarray, h: int, sigmas: np.ndarray) -> np.ndarray:
    """xb: [N, D] batch slice; h: column parity (0=even, 1=odd)."""
    out = np.zeros((128, XC_W), dtype=np.float32)
    xbt = xb.T  # [D, N]
    if h == 0:
        out[:, :XI_OFF] = xbt.reshape(1, D * N)
    else:
        shifted = np.zeros((D, N), dtype=np.float32)
        shifted[:, :N - 1] = xbt[:, 1:]
        out[:, :XI_OFF] = shifted.reshape(1, D * N)
    rows = xb.reshape(NT, 128, D)            # [r, p, d]
    out[:, XI_OFF:SIG_OFF] = rows.transpose(1, 0, 2).reshape(128, NT * D)
    out[:, SIG_OFF:] = sigmas[None, :]
    return out


def kernel(x: np.ndarray, sigmas: np.ndarray) -> np.ndarray:
    global _cached, LAST_RESULT
    from concourse import bass_utils

    x = np.ascontiguousarray(np.asarray(x, dtype=np.float32))
    sigmas = np.ascontiguousarray(np.asarray(sigmas, dtype=np.float32))

    if _cached is None:
        _cached = _build()
    nc = _cached

    in_maps = []
    for c in range(NCORES):
        b, h = c // 2, c % 2
        in_maps.append({"xc": _pack_core_input(x[b], h, sigmas)})

    res = bass_utils.run_bass_kernel_spmd(
        nc, in_maps, core_ids=list(range(NCORES)), **TRACE_KW
    )
    LAST_RESULT = res

    out = np.empty((B, S, N, N), dtype=np.float32)
    for c in range(NCORES):
        b, h = c // 2, c % 2
        yl = np.asarray(res.results[c]["y"]).astype(np.float32)  # [NG,S,128,GW]
        for g, grp in enumerate(GROUPS):
            off = 0
            for r in grp:
                w = HW[r]
                c0 = 128 * r + h
                out[b, :, r * 128:(r + 1) * 128, c0:c0 + 2 * w:2] = (
                    yl[g, :, :, off:off + w]
                )
                off += w
    # mirror the lower triangle (bit-exact by symmetry)
    for r in range(NT - 1):
        src = out[:, :, r * 128:(r + 1) * 128, (r + 1) * 128:]
        out[:, :, (r + 1) * 128:, r * 128:(r + 1) * 128] = src.swapaxes(-1, -2)
    return out
